# revision 5
# baseline (speedup 1.0000x reference)
"""Multi-head attention block (QKV proj -> attention -> out proj -> residual+LN)
for Trainium2, sharded over 8 NeuronCores.

Sharding: (batch b, query-row quarter g) -> core c = b*4 + g. Each core computes
full K/V projections for its batch (replicated within the 4-core group, which
avoids all cross-core communication), then all 16 heads of attention for its
512 query rows, the output projection, residual add and LayerNorm.

V is projected to a DRAM scratch buffer in phase 1 and streamed back per head
in phase 2 (SBUF cannot hold K^T and V at once alongside the working set).

All matmuls run in float32r (fast fp32 mode on the PE array, ~1e-4 rel err).
"""
import sys

import numpy as np

for _p in ("/opt/trn_rl_repo",):
    if _p not in sys.path:
        sys.path.insert(0, _p)

from contextlib import ExitStack

import concourse.bacc as bacc
import concourse.tile as tile
from concourse import mybir
from concourse.bass_utils import run_bass_kernel_spmd
from concourse.masks import make_identity

P = 128
B = 2
S = 2048
D = 1024
H = 16
DK = 64
SQ = 512            # query rows per core
NQT = SQ // P       # 4 q-tiles
NKT = D // P        # 8 contraction tiles over D
NTT = S // P        # 16 t-tiles
NHP = H // 2        # 8 head pairs
N_CORES = 8
SCALE = float(1.0 / np.sqrt(DK))
LN_EPS = 1e-5

f32 = mybir.dt.float32
f32r = mybir.dt.float32r
FT = mybir.ActivationFunctionType
ALU = mybir.AluOpType

_CACHE = {}


def _build():
    nc = bacc.Bacc("TRN2", target_bir_lowering=False, debug=False,
                   num_devices=N_CORES)

    xq_d = nc.dram_tensor("xq", [SQ, D], f32, kind="ExternalInput").ap()
    xk_d = nc.dram_tensor("xk", [S, D], f32, kind="ExternalInput").ap()
    xv_d = nc.dram_tensor("xv", [S, D], f32, kind="ExternalInput").ap()
    wq_d = nc.dram_tensor("wq", [D, D], f32, kind="ExternalInput").ap()
    wk_d = nc.dram_tensor("wk", [D, D], f32, kind="ExternalInput").ap()
    wv_d = nc.dram_tensor("wv", [D, D], f32, kind="ExternalInput").ap()
    wo_d = nc.dram_tensor("wo", [D, D], f32, kind="ExternalInput").ap()
    bq_d = nc.dram_tensor("bq", [1, D], f32, kind="ExternalInput").ap()
    bk_d = nc.dram_tensor("bk", [1, D], f32, kind="ExternalInput").ap()
    bv_d = nc.dram_tensor("bv", [1, D], f32, kind="ExternalInput").ap()
    bo_d = nc.dram_tensor("bo", [1, D], f32, kind="ExternalInput").ap()
    gamma_d = nc.dram_tensor("gamma", [1, D], f32, kind="ExternalInput").ap()
    beta_d = nc.dram_tensor("beta", [1, D], f32, kind="ExternalInput").ap()
    attn_d = nc.dram_tensor("attn", [H, SQ, S], f32, kind="ExternalOutput").ap()
    y_d = nc.dram_tensor("y", [SQ, D], f32, kind="ExternalOutput").ap()
    # internal DRAM scratch for V, head-major for contiguous per-head reads
    v_d = nc.dram_tensor("v_scratch", [H, S, DK], f32).ap()

    wq_r = wq_d.rearrange("(t p) o -> p t o", p=P)
    wk_r = wk_d.rearrange("(t p) o -> p t o", p=P)
    wv_r = wv_d.rearrange("(t p) o -> p t o", p=P)
    wo_r = wo_d.rearrange("(t p) o -> p t o", p=P)

    with ExitStack() as ctx:
        tc = ctx.enter_context(tile.TileContext(nc))
        # SBUF pools (per-partition bytes are the scarce resource: 224 KB)
        consts = ctx.enter_context(tc.tile_pool(name="consts", bufs=1))   # ~2.2 KB
        xT_pool = ctx.enter_context(tc.tile_pool(name="xT", bufs=1))      # 64 KB
        kT_pool = ctx.enter_context(tc.tile_pool(name="kT", bufs=1))      # 64 KB
        qT_pool = ctx.enter_context(tc.tile_pool(name="qTp", bufs=1))     # 16 KB
        t2_pool = ctx.enter_context(tc.tile_pool(name="t2", bufs=1))      # 16 KB
        wch_pool = ctx.enter_context(tc.tile_pool(name="wch", bufs=1))    # 8 KB
        xrow_pool = ctx.enter_context(tc.tile_pool(name="xrow", bufs=2))  # 8 KB
        e_pool = ctx.enter_context(tc.tile_pool(name="epool", bufs=2))    # 8 KB
        et_pool = ctx.enter_context(tc.tile_pool(name="etpool", bufs=3))  # 6 KB
        vh_pool = ctx.enter_context(tc.tile_pool(name="vh", bufs=2))      # 8 KB
        row_pool = ctx.enter_context(tc.tile_pool(name="rows", bufs=2))   # 4 KB
        ve_pool = ctx.enter_context(tc.tile_pool(name="vev", bufs=4))     # 1 KB
        tiny = ctx.enter_context(tc.tile_pool(name="tiny", bufs=4))       # ~1 KB
        ps_big = ctx.enter_context(tc.tile_pool(name="ps_big", bufs=2, space="PSUM"))
        ps_small = ctx.enter_context(tc.tile_pool(name="ps_small", bufs=2, space="PSUM"))
        ps_ctx = ctx.enter_context(tc.tile_pool(name="ps_ctx", bufs=2, space="PSUM"))

        # ---- constants ----
        ident = consts.tile([P, P], f32, tag="ident")
        make_identity(nc, ident)
        ones_f = consts.tile([1, P], f32, tag="ones_f")
        nc.vector.memset(ones_f, 1.0)
        ones_r = consts.tile([1, P], f32r, tag="ones")
        nc.scalar.copy(out=ones_r, in_=ones_f)
        bq_col = consts.tile([P, NKT], f32, tag="bq_col")
        nc.sync.dma_start(out=bq_col, in_=bq_d.rearrange("o (t p) -> p (o t)", p=P))
        bk_col = consts.tile([P, NKT], f32, tag="bk_col")
        nc.sync.dma_start(out=bk_col, in_=bk_d.rearrange("o (t p) -> p (o t)", p=P))
        eps_t = consts.tile([P, 1], f32, tag="eps")
        nc.vector.memset(eps_t, LN_EPS)

        def transpose_x(x_dram, nrt, dst):
            # x_dram [nrt*128, D] row-major -> dst [P, NKT, nrt*128] = x^T
            for rt in range(nrt):
                xrow = xrow_pool.tile([P, D], f32, tag="xrow")
                nc.sync.dma_start(out=xrow, in_=x_dram[rt * P:(rt + 1) * P, :])
                for kt in range(NKT):
                    pst = ps_small.tile([P, P], f32, tag="ps_s")
                    nc.tensor.transpose(pst, xrow[:, kt * P:(kt + 1) * P], ident)
                    nc.scalar.copy(out=dst[:, kt, rt * P:(rt + 1) * P], in_=pst)

        # ---- K^T = (xk @ Wk + bk)^T, layout [dk(head pair), hp, t] ----
        xkT = xT_pool.tile([P, NKT, S], f32r, tag="xT")
        transpose_x(xk_d, NTT, xkT)
        kT = kT_pool.tile([P, NHP, S], f32r, tag="kT")
        for m in range(NHP):
            wkc = wch_pool.tile([P, NKT, P], f32r, tag="wch")
            nc.sync.dma_start(
                out=wkc, in_=wk_r[:, :, m * P:(m + 1) * P].bitcast(f32r))
            for n in range(S // SQ):
                psk = ps_small.tile([P, SQ], f32, tag="ps_s")
                for kt in range(NKT):
                    nc.tensor.matmul(psk, lhsT=wkc[:, kt, :],
                                     rhs=xkT[:, kt, n * SQ:(n + 1) * SQ],
                                     start=(kt == 0), stop=(kt == NKT - 1))
                nc.scalar.activation(out=kT[:, m, n * SQ:(n + 1) * SQ], in_=psk,
                                     func=FT.Identity,
                                     bias=bk_col[:, m:m + 1], scale=1.0)

        # ---- V = xv @ Wv + bv -> DRAM scratch [H, S, DK] ----
        bv_row = e_pool.tile([1, D], f32r, tag="E", name="bv_row")
        nc.sync.dma_start(out=bv_row, in_=bv_d.bitcast(f32r))
        xvT = xT_pool.tile([P, NKT, S], f32r, tag="xT")
        transpose_x(xv_d, NTT, xvT)
        for n in range(4):                      # out-dim chunks of 256 (4 heads)
            wvc = wch_pool.tile([P, NKT, 256], f32r, tag="wch")
            nc.sync.dma_start(
                out=wvc, in_=wv_r[:, :, n * 256:(n + 1) * 256].bitcast(f32r))
            for tt in range(NTT):
                psv = ps_small.tile([P, 256], f32, tag="ps_s")
                nc.tensor.matmul(psv, lhsT=ones_r[0:1, 0:P],
                                 rhs=bv_row[0:1, n * 256:(n + 1) * 256],
                                 start=True, stop=False)
                for kt in range(NKT):
                    nc.tensor.matmul(psv, lhsT=xvT[:, kt, tt * P:(tt + 1) * P],
                                     rhs=wvc[:, kt, :],
                                     start=False, stop=(kt == NKT - 1))
                for j in range(4):              # per-head 32 KB contiguous
                    h = n * 4 + j
                    vev = ve_pool.tile([P, DK], f32, tag="vev")
                    nc.scalar.copy(out=vev, in_=psv[:, j * DK:(j + 1) * DK])
                    nc.sync.dma_start(out=v_d[h, tt * P:(tt + 1) * P, :],
                                      in_=vev)

        # ---- Q^T = (xq @ Wq + bq)^T, layout [dk(head pair), hp, q] ----
        xqT = xT_pool.tile([P, NKT, SQ], f32r, tag="xT")
        transpose_x(xq_d, NQT, xqT)
        qT = qT_pool.tile([P, NHP, SQ], f32r, tag="qT")
        for m in range(NHP):
            wqc = wch_pool.tile([P, NKT, P], f32r, tag="wch")
            nc.sync.dma_start(
                out=wqc, in_=wq_r[:, :, m * P:(m + 1) * P].bitcast(f32r))
            psq = ps_small.tile([P, SQ], f32, tag="ps_s")
            for kt in range(NKT):
                nc.tensor.matmul(psq, lhsT=wqc[:, kt, :], rhs=xqT[:, kt, :],
                                 start=(kt == 0), stop=(kt == NKT - 1))
            nc.scalar.activation(out=qT[:, m, :], in_=psq, func=FT.Identity,
                                 bias=bq_col[:, m:m + 1], scale=1.0)

        # ---- attention ----
        ctxT = t2_pool.tile([P, NHP, SQ], f32r, tag="t2")
        for h in range(H):
            hp, hh = divmod(h, 2)
            psl = slice(hh * DK, (hh + 1) * DK)
            vht = vh_pool.tile([P, NTT, DK], f32r, tag="vh")
            nc.sync.dma_start(
                out=vht,
                in_=v_d[h].rearrange("(tt p) d -> p tt d", p=P).bitcast(f32r))
            recip_row = row_pool.tile([1, SQ], f32r, tag="rows")
            # S path: scores [q, t], softmax over free dim, attn out
            for qt in range(NQT):
                Eh = [e_pool.tile([P, 1024], f32, tag="E", name=f"E{i}")
                      for i in range(2)]
                sums = tiny.tile([P, 2], f32, tag="sums")
                for half in range(2):
                    psS = ps_big.tile([P, 1024], f32, tag="ps_big")
                    for n in range(2):
                        t0 = half * 1024 + n * 512
                        nc.tensor.matmul(
                            psS[:, n * 512:(n + 1) * 512],
                            lhsT=qT[psl, hp, qt * P:(qt + 1) * P],
                            rhs=kT[psl, hp, t0:t0 + 512],
                            start=True, stop=True)
                    nc.scalar.activation(
                        out=Eh[half], in_=psS, func=FT.Exp, scale=SCALE,
                        accum_out=sums[:, half:half + 1])
                ssum = tiny.tile([P, 1], f32, tag="ssum")
                nc.vector.tensor_add(ssum, sums[:, 0:1], sums[:, 1:2])
                rec = tiny.tile([P, 1], f32, tag="rec")
                nc.vector.reciprocal(rec, ssum)
                for half in range(2):
                    nc.vector.tensor_scalar_mul(Eh[half], Eh[half], rec)
                    nc.sync.dma_start(
                        out=attn_d[h, qt * P:(qt + 1) * P,
                                   half * 1024:(half + 1) * 1024],
                        in_=Eh[half])
                # transpose recip into the row layout the ctx scaling needs
                psr = ps_small.tile([1, P], f32, tag="ps_s")
                nc.tensor.transpose(psr, rec, ident)
                nc.scalar.copy(out=recip_row[0:1, qt * P:(qt + 1) * P],
                               in_=psr)
            # S^T path: scores [t, q] -> exp -> ctx^T accumulation
            psC = ps_ctx.tile([DK, SQ], f32, tag="ps_c")
            for tt in range(NTT):
                psT = ps_small.tile([P, SQ], f32, tag="ps_s")
                nc.tensor.matmul(psT, lhsT=kT[psl, hp, tt * P:(tt + 1) * P],
                                 rhs=qT[psl, hp, :], start=True, stop=True)
                ET = et_pool.tile([P, SQ], f32r, tag="ET")
                nc.scalar.activation(out=ET, in_=psT, func=FT.Exp, scale=SCALE)
                nc.tensor.matmul(psC, lhsT=vht[:, tt, :], rhs=ET,
                                 start=(tt == 0), stop=(tt == NTT - 1))
            # ctx^T[h] = psC * recip (broadcast along partitions via rank-1)
            psR = ps_small.tile([DK, SQ], f32, tag="ps_s")
            nc.tensor.matmul(psR, lhsT=ones_r[0:1, 0:DK], rhs=recip_row,
                             start=True, stop=True)
            RbS = et_pool.tile([DK, SQ], f32, tag="ET", name="RbS")
            nc.scalar.copy(out=RbS, in_=psR)
            nc.vector.tensor_mul(ctxT[psl, hp, :], psC, RbS)

        # ---- out projection + residual + LayerNorm ----
        bo_b = vh_pool.tile([P, D], f32, tag="vh", name="bo_b")
        nc.sync.dma_start(out=bo_b, in_=bo_d.to_broadcast([P, D]))
        gamma_b = e_pool.tile([P, D], f32, tag="E", name="gamma_b")
        nc.sync.dma_start(out=gamma_b, in_=gamma_d.to_broadcast([P, D]))
        beta_b = e_pool.tile([P, D], f32, tag="E", name="beta_b")
        nc.sync.dma_start(out=beta_b, in_=beta_d.to_broadcast([P, D]))
        wof = xT_pool.tile([P, NKT, D], f32r, tag="xT")
        nc.sync.dma_start(out=wof, in_=wo_r.bitcast(f32r))
        for qt in range(NQT):
            xrow = xrow_pool.tile([P, D], f32, tag="xrow")
            nc.sync.dma_start(out=xrow, in_=xq_d[qt * P:(qt + 1) * P, :])
            xqb = xrow_pool.tile([P, D], f32, tag="xrow")
            nc.gpsimd.tensor_add(xqb, xrow, bo_b)
            psO = ps_big.tile([P, 1024], f32, tag="ps_big")
            for n in range(2):
                for kt in range(NKT):
                    nc.tensor.matmul(psO[:, n * 512:(n + 1) * 512],
                                     lhsT=ctxT[:, kt, qt * P:(qt + 1) * P],
                                     rhs=wof[:, kt, n * 512:(n + 1) * 512],
                                     start=(kt == 0), stop=(kt == NKT - 1))
            nc.vector.tensor_add(xqb, psO, xqb)
            # LayerNorm over the 1024-wide free dim
            stats = tiny.tile([P, 2, 6], f32, tag="stats")
            for sg in range(2):
                nc.vector.bn_stats(out=stats[:, sg, :],
                                   in_=xqb[:, sg * 512:(sg + 1) * 512])
            mv = tiny.tile([P, 2], f32, tag="mv")
            nc.vector.bn_aggr(out=mv, in_=stats)
            rstd = tiny.tile([P, 1], f32, tag="rstd")
            nc.scalar.activation(out=rstd, in_=mv[:, 1:2], func=FT.Sqrt,
                                 bias=eps_t, scale=1.0)
            nc.vector.reciprocal(rstd, rstd)
            nc.vector.tensor_scalar(xqb, xqb, mv[:, 0:1], rstd,
                                    op0=ALU.subtract, op1=ALU.mult)
            nc.vector.tensor_mul(xqb, xqb, gamma_b)
            nc.vector.tensor_add(xqb, xqb, beta_b)
            nc.sync.dma_start(out=y_d[qt * P:(qt + 1) * P, :], in_=xqb)

    nc.compile()
    return nc


def _get_nc():
    if "nc" not in _CACHE:
        _CACHE["nc"] = _build()
    return _CACHE["nc"]


def kernel(query, key, value, Wq, bq, Wk, bk, Wv, bv, Wo, bo, gamma, beta):
    nc = _get_nc()
    f = lambda a: np.ascontiguousarray(np.asarray(a, dtype=np.float32))
    query, key, value = f(query), f(key), f(value)
    shared = {
        "wq": f(Wq), "wk": f(Wk), "wv": f(Wv), "wo": f(Wo),
        "bq": f(bq).reshape(1, D), "bk": f(bk).reshape(1, D),
        "bv": f(bv).reshape(1, D), "bo": f(bo).reshape(1, D),
        "gamma": f(gamma).reshape(1, D), "beta": f(beta).reshape(1, D),
    }
    in_maps = []
    for c in range(N_CORES):
        b, g = divmod(c, 4)
        in_maps.append({
            "xq": f(query[b, g * SQ:(g + 1) * SQ]),
            "xk": key[b], "xv": value[b], **shared,
        })
    res = run_bass_kernel_spmd(nc, in_maps, list(range(N_CORES))).results
    y = np.empty((B, S, D), np.float32)
    attn = np.empty((B, H, S, S), np.float32)
    for c in range(N_CORES):
        b, g = divmod(c, 4)
        y[b, g * SQ:(g + 1) * SQ] = res[c]["y"]
        attn[b, :, g * SQ:(g + 1) * SQ, :] = res[c]["attn"]
    return y, attn
